# revision 2
# baseline (speedup 1.0000x reference)
"""Cumulative-FFT Trainium2 kernel.

out[b,t,d,k,c] = pos_norm[t] * cumsum_t( x[b,t,d] * twiddles[t,k,c] )

Shapes (hardcoded): x (4,1024,512) bf16, twiddles (1024,32,2) bf16,
pos_norm (1024,) bf16  ->  out (4,1024,512,32,2) bf16.

Sharding: 8 cores = batch(4) x d_model-half(2). Each core computes a
(1024, 256*64) bf16 shard (32 MiB) -- data-parallel over B, tensor-parallel
over D, nothing crosses cores.

Per-core algorithm: the cumsum along t is done as a per-block triangular
matmul on the TensorEngine. t is split into blocks of 127 rows; the moving
operand c holds the bf16 contributions c[s, kc*256+d] = x[s,d]*tw[s,kc]
(one 2x-mode DVE tensor_tensor against a 16x-replicated tw tile) plus one
extra row (s = L) holding the carry = column sums of all previous blocks
(maintained by a tiny tw^T @ x matmul per block). The stationary operand
folds both the causal mask and the pos_norm[t] scale:

    utri[s, t] = pos_norm[t0+t] * (1 if (s <= t or s == L) else 0)

so  psum[t, n] = pos[t] * (carry[n] + sum_{s<=t} c[s, n])  comes out of the
matmul fully finished; eviction to SBUF is a pure fp32->bf16 copy split
between VectorE and ScalarE into ONE [128, 16384] staging tile per block,
then a single ~4 MiB contiguous HWDGE store per block (alternating the
qSync/qScalar HW-DGE queues) writes the shard.

The v1 kernel issued 11 small stores per block, 9/11 of them on the gpsimd
SWDGE queue; the NTFF trace showed that queue poisoned by ~15 eight-byte
bookkeeping packets per data packet (68k packets, ~35% of SDMA engine time)
and aggregate store bandwidth of ~63 GB/s. HWDGE queues showed none of
that, so all bulk traffic now goes HWDGE with 32 KiB/partition descriptors.
"""

import sys

sys.path.insert(0, "/opt/trn_rl_repo")

import ml_dtypes
import numpy as np

import concourse.bass as bass
import concourse.mybir as mybir
import concourse.tile as tile
from concourse import bacc
import concourse.bass_utils as _bu
from concourse.bass_utils import run_bass_kernel_spmd

# note: walrus --enable-ldw-opt=true crashes codegen (visitInstLdweights),
# so the per-matmul LDWEIGHTS reload cannot be elided

B, T, D = 4, 1024, 512
KC = 64            # 32 freqs x (cos,sin), flattened innermost dims of out
DSH = D // 2       # d-slice per core
NKC = DSH * KC     # free elements per t per core (16384)
BLK = 127          # data rows per t-block; row L is the carry row
NBLK = (T + BLK - 1) // BLK  # 9 (8 x 127 + 1 x 8)

BF16 = mybir.dt.bfloat16
F32 = mybir.dt.float32

# groups of consecutive 512-wide matmul tiles evicted by one copy op
_EVICT_GROUPS = [(g * 3, min(3, 32 - g * 3)) for g in range((32 + 2) // 3)]
_DVE_GROUPS = (0, 4, 8)  # eviction groups handled by VectorE (rest ScalarE)

LAST_RESULTS = None  # set by kernel(); test.py reads exec_time_ns from here


def _build_utri(pos_norm: np.ndarray) -> np.ndarray:
    """Stationary operands for all blocks, packed (128, NBLK*128) bf16."""
    pos = np.asarray(pos_norm).astype(np.float32)
    utri = np.zeros((128, NBLK * 128), np.float32)
    s = np.arange(128)[:, None]
    for k in range(NBLK):
        t0 = k * BLK
        L = min(BLK, T - t0)
        t = np.arange(L)[None, :]
        mask = ((s < L) & (s <= t)) | (s == L)
        utri[:, 128 * k : 128 * k + L] = mask * pos[t0 : t0 + L][None, :]
    return utri.astype(ml_dtypes.bfloat16)


def _build_program() -> bass.Bass:
    nc = bacc.Bacc("TRN2", target_bir_lowering=False, debug=False)
    x_d = nc.dram_tensor("x_shard", [T, DSH], BF16, kind="ExternalInput").ap()
    tw_d = nc.dram_tensor("tw", [T, KC], BF16, kind="ExternalInput").ap()
    utri_d = nc.dram_tensor("utri", [128, NBLK * 128], BF16, kind="ExternalInput").ap()
    out_d = nc.dram_tensor("out_shard", [T, NKC], BF16, kind="ExternalOutput").ap()

    with tile.TileContext(nc) as tc:
        with (
            tc.tile_pool(name="singles", bufs=1) as singles,
            tc.tile_pool(name="xp", bufs=3) as xp,
            tc.tile_pool(name="twp", bufs=3) as twp,
            tc.tile_pool(name="cp", bufs=3) as cp,
            tc.tile_pool(name="outp", bufs=2) as outp,
            tc.tile_pool(name="repp", bufs=2) as repp,
            tc.tile_pool(name="carryp", bufs=3) as carryp,
            tc.tile_pool(name="pmain", bufs=2, space="PSUM") as pmain,
            tc.tile_pool(name="pdelta", bufs=1, space="PSUM") as pdelta,
            tc.tile_pool(name="pwarm", bufs=1, space="PSUM") as pwarm,
        ):
            utri_sb = singles.tile([128, NBLK * 128], BF16)
            nc.sync.dma_start(out=utri_sb[:, :], in_=utri_d[:, :])
            carry_zero = singles.tile([KC, DSH], BF16)
            nc.vector.memset(carry_zero[:, :], 0.0)
            # warmup operand with no load dependency so the PE HAM trip
            # starts at t=0, concurrent with the utri load
            wz = singles.tile([128, DSH], BF16)
            nc.gpsimd.memset(wz[:, :], 1.0)

            # ~6us of back-to-back dummy matmuls: trips the PE HAM activity
            # monitor so the real matmuls run at 2.4 GHz instead of 1.2
            warm_ps = pwarm.tile([KC, DSH], F32)
            for _ in range(28):
                nc.tensor.matmul(
                    warm_ps[:, :],
                    lhsT=wz[:128, 0:KC],
                    rhs=wz[:128, 0:DSH],
                    start=True, stop=True,
                )

            carry_prev = carry_zero
            for k in range(NBLK):
                t0 = k * BLK
                L = min(BLK, T - t0)

                x_sb = xp.tile([128, DSH], BF16)
                nc.sync.dma_start(out=x_sb[:L, :], in_=x_d[t0 : t0 + L, :])
                tw_sb = twp.tile([128, KC], BF16)
                nc.sync.dma_start(out=tw_sb[:L, :], in_=tw_d[t0 : t0 + L, :])

                # contributions, kc-major: c[s, kc*DSH + d] = x[s,d] * tw[s,kc]
                # as ONE bf16 tensor_tensor in the DVE 2x mode. The tw operand
                # streams from a 16x-replicated tile (built by log-doubling
                # copies on the otherwise-idle GpSimd engine) through a 4-D AP
                # whose innermost dim has stride 1 -- a 0-stride dim anywhere
                # closer in would demote the op to 1x, and a per-kc
                # tensor_scalar is stuck at 1x too (its scalar operand must be
                # fp32).
                rep16 = repp.tile([128, KC * 16], BF16)
                r16v = rep16.rearrange("p (a c) -> p a c", c=16)
                nc.gpsimd.tensor_copy(r16v[:L, :, 0:1], tw_sb[:L, :, None])
                w = 1
                while w < 16:
                    nc.gpsimd.tensor_copy(r16v[:L, :, w : 2 * w], r16v[:L, :, 0:w])
                    w *= 2
                c_sb = cp.tile([128, NKC], BF16)
                c_v = c_sb[:L, :].rearrange("p (a b c) -> p a b c", b=16, c=16)
                x_v = (
                    x_sb[:L, :]
                    .rearrange("p (b c) -> p b c", c=16)
                    .unsqueeze(1)
                    .broadcast_to((L, KC, 16, 16))
                )
                rep_v = (
                    rep16[:L, :]
                    .rearrange("p (a c) -> p a c", c=16)
                    .unsqueeze(2)
                    .broadcast_to((L, KC, 16, 16))
                )
                nc.vector.tensor_mul(c_v, x_v, rep_v)
                # carry row: flattened (kc, d) sums over all previous blocks
                nc.gpsimd.dma_start(out=c_sb[L : L + 1, :], in_=carry_prev[:, :])

                # carry for the next block: += tw_k^T @ x_k
                if k + 1 < NBLK:
                    delta = pdelta.tile([KC, DSH], F32)
                    nc.tensor.matmul(
                        delta[:, :], lhsT=tw_sb[:L, :], rhs=x_sb[:L, :],
                        start=True, stop=True,
                    )
                    carry_new = carryp.tile([KC, DSH], BF16)
                    if k == 0:
                        nc.vector.tensor_copy(carry_new[:, :], delta[:, :])
                    else:
                        nc.vector.tensor_add(
                            carry_new[:, :], carry_prev[:, :], delta[:, :]
                        )
                    carry_prev = carry_new

                # all 11 eviction groups land in ONE staging tile (disjoint
                # column ranges from two engines; subtile dep tracking keeps
                # them concurrent), so the whole block ships as a single
                # contiguous ~4 MiB HWDGE store with 32 KiB descriptors

                # full 128-column stationary (cols >= L are zero-padded in
                # utri) so walrus enables FWL on the LDWEIGHTS
                lhsT = utri_sb[: L + 1, 128 * k : 128 * (k + 1)]
                og = outp.tile([128, NKC], BF16)
                for gi, (j0, gn) in enumerate(_EVICT_GROUPS):
                    pg = pmain.tile([128, 1536], F32)
                    for jj in range(gn):
                        j = j0 + jj
                        nc.tensor.matmul(
                            pg[:, jj * 512 : (jj + 1) * 512],
                            lhsT=lhsT,
                            rhs=c_sb[: L + 1, j * 512 : (j + 1) * 512],
                            start=True, stop=True,
                        )
                    col = j0 * 512
                    if gi in _DVE_GROUPS:
                        nc.vector.tensor_copy(
                            og[:L, col : col + gn * 512], pg[:L, : gn * 512]
                        )
                    else:
                        nc.scalar.copy(
                            og[:L, col : col + gn * 512], pg[:L, : gn * 512]
                        )
                # one store per block; alternate the two HW-DGE queues so
                # consecutive blocks' stores drain concurrently
                eng = nc.sync if k % 2 == 0 else nc.scalar
                eng.dma_start(out=out_d[t0 : t0 + L, :], in_=og[:L, :])
    nc.compile()
    return nc


def kernel(**inputs) -> np.ndarray:
    global LAST_RESULTS
    x = np.asarray(inputs["x"])                       # (4,1024,512) bf16
    tw = np.asarray(inputs["twiddles"])               # (1024,32,2) bf16
    pos = np.asarray(inputs["pos_norm"])              # (1024,) bf16

    tw2 = np.ascontiguousarray(tw.reshape(T, KC))
    utri = _build_utri(pos)

    in_maps = []
    for core in range(8):
        b, dh = core // 2, core % 2
        xs = np.ascontiguousarray(x[b, :, dh * DSH : (dh + 1) * DSH])
        in_maps.append({"x_shard": xs, "tw": tw2, "utri": utri})

    nc = _build_program()
    res = run_bass_kernel_spmd(nc, in_maps, core_ids=list(range(8)))
    LAST_RESULTS = res

    out = np.empty((B, T, D, KC // 2, 2), dtype=x.dtype)
    for core in range(8):
        b, dh = core // 2, core % 2
        o = np.asarray(res.results[core]["out_shard"])  # (T, NKC) kc-major
        o = o.reshape(T, KC, DSH).transpose(0, 2, 1)    # -> (T, DSH, KC)
        out[b, :, dh * DSH : (dh + 1) * DSH, :, :] = o.reshape(T, DSH, KC // 2, 2)
    return out


if __name__ == "__main__":
    rng = np.random.default_rng(0)
    demo = {
        "x": rng.standard_normal((B, T, D), np.float32).astype(ml_dtypes.bfloat16),
        "twiddles": rng.standard_normal((T, KC // 2, 2), np.float32).astype(
            ml_dtypes.bfloat16
        ),
        "pos_norm": (1.0 / np.sqrt(np.arange(1, T + 1, dtype=np.float32))).astype(
            ml_dtypes.bfloat16
        ),
    }
    print(kernel(**demo).shape)


# revision 3
# speedup vs baseline: 5.2481x; 5.2481x over previous
"""Cumulative-FFT Trainium2 kernel.

out[b,t,d,k,c] = pos_norm[t] * cumsum_t( x[b,t,d] * twiddles[t,k,c] )

Shapes (hardcoded): x (4,1024,512) bf16, twiddles (1024,32,2) bf16,
pos_norm (1024,) bf16  ->  out (4,1024,512,32,2) bf16.

Sharding: 8 cores = batch(4) x d_model-half(2). Each core computes a
(1024, 256*64) bf16 shard (32 MiB) -- data-parallel over B, tensor-parallel
over D, nothing crosses cores.

Per-core algorithm: the cumsum along t is done as a per-block triangular
matmul on the TensorEngine. t is split into blocks of 127 rows; the moving
operand c holds the bf16 contributions c[s, kc*256+d] = x[s,d]*tw[s,kc]
(one 2x-mode DVE tensor_tensor against a 16x-replicated tw tile) plus one
extra row (s = L) holding the carry = column sums of all previous blocks
(maintained by a tiny tw^T @ x matmul per block). The stationary operand
folds both the causal mask and the pos_norm[t] scale:

    utri[s, t] = pos_norm[t0+t] * (1 if (s <= t or s == L) else 0)

so  psum[t, n] = pos[t] * (carry[n] + sum_{s<=t} c[s, n])  comes out of the
matmul fully finished; eviction to SBUF is a pure fp32->bf16 copy split
between VectorE and ScalarE into ONE [128, 16384] staging tile per block,
then a single ~4 MiB contiguous HWDGE store per block (alternating the
qSync/qScalar HW-DGE queues) writes the shard.

The v1 kernel issued 11 small stores per block, 9/11 of them on the gpsimd
SWDGE queue; the NTFF trace showed that queue poisoned by ~15 eight-byte
bookkeeping packets per data packet (68k packets, ~35% of SDMA engine time)
and aggregate store bandwidth of ~63 GB/s. HWDGE queues showed none of
that, so all bulk traffic now goes HWDGE with 32 KiB/partition descriptors.
"""

import sys

sys.path.insert(0, "/opt/trn_rl_repo")

import ml_dtypes
import numpy as np

import concourse.bass as bass
import concourse.mybir as mybir
import concourse.tile as tile
from concourse import bacc
import concourse.bass_utils as _bu
from concourse.bass_utils import run_bass_kernel_spmd

# note: walrus --enable-ldw-opt=true crashes codegen (visitInstLdweights),
# so the per-matmul LDWEIGHTS reload cannot be elided

B, T, D = 4, 1024, 512
KC = 64            # 32 freqs x (cos,sin), flattened innermost dims of out
DSH = D // 2       # d-slice per core
NKC = DSH * KC     # free elements per t per core (16384)
BLK = 112          # data rows per t-block (mult of 16: HWDGE stripes a DMA
                   # across the 16 SDMA engines only when the partition count
                   # divides by 16); row L is the carry row
NBLK = (T + BLK - 1) // BLK  # 10 (9 x 112 + 1 x 16)

BF16 = mybir.dt.bfloat16
F32 = mybir.dt.float32

# groups of consecutive 512-wide matmul tiles evicted by one copy op
_EVICT_GROUPS = [(g * 3, min(3, 32 - g * 3)) for g in range((32 + 2) // 3)]
_DVE_GROUPS = (0, 4, 8)  # eviction groups handled by VectorE (rest ScalarE)

LAST_RESULTS = None  # set by kernel(); test.py reads exec_time_ns from here


def _build_utri(pos_norm: np.ndarray) -> np.ndarray:
    """Stationary operands for all blocks, packed (128, NBLK*128) bf16."""
    pos = np.asarray(pos_norm).astype(np.float32)
    utri = np.zeros((128, NBLK * 128), np.float32)
    s = np.arange(128)[:, None]
    for k in range(NBLK):
        t0 = k * BLK
        L = min(BLK, T - t0)
        t = np.arange(L)[None, :]
        mask = ((s < L) & (s <= t)) | (s == L)
        utri[:, 128 * k : 128 * k + L] = mask * pos[t0 : t0 + L][None, :]
    return utri.astype(ml_dtypes.bfloat16)


def _build_program() -> bass.Bass:
    nc = bacc.Bacc("TRN2", target_bir_lowering=False, debug=False)
    x_d = nc.dram_tensor("x_shard", [T, DSH], BF16, kind="ExternalInput").ap()
    tw_d = nc.dram_tensor("tw", [T, KC], BF16, kind="ExternalInput").ap()
    utri_d = nc.dram_tensor("utri", [128, NBLK * 128], BF16, kind="ExternalInput").ap()
    out_d = nc.dram_tensor("out_shard", [T, NKC], BF16, kind="ExternalOutput").ap()

    with tile.TileContext(nc) as tc:
        with (
            tc.tile_pool(name="singles", bufs=1) as singles,
            tc.tile_pool(name="xp", bufs=3) as xp,
            tc.tile_pool(name="twp", bufs=3) as twp,
            tc.tile_pool(name="cp", bufs=3) as cp,
            tc.tile_pool(name="outp", bufs=2) as outp,
            tc.tile_pool(name="repp", bufs=2) as repp,
            tc.tile_pool(name="carryp", bufs=3) as carryp,
            tc.tile_pool(name="pmain", bufs=2, space="PSUM") as pmain,
            tc.tile_pool(name="pdelta", bufs=1, space="PSUM") as pdelta,
            tc.tile_pool(name="pwarm", bufs=1, space="PSUM") as pwarm,
        ):
            utri_sb = singles.tile([128, NBLK * 128], BF16)
            nc.sync.dma_start(out=utri_sb[:, :], in_=utri_d[:, :])
            carry_zero = singles.tile([KC, DSH], BF16)
            nc.vector.memset(carry_zero[:, :], 0.0)
            # warmup operand with no load dependency so the PE HAM trip
            # starts at t=0, concurrent with the utri load
            wz = singles.tile([128, DSH], BF16)
            nc.gpsimd.memset(wz[:, :], 1.0)

            # ~6us of back-to-back dummy matmuls: trips the PE HAM activity
            # monitor so the real matmuls run at 2.4 GHz instead of 1.2
            warm_ps = pwarm.tile([KC, DSH], F32)
            for _ in range(28):
                nc.tensor.matmul(
                    warm_ps[:, :],
                    lhsT=wz[:128, 0:KC],
                    rhs=wz[:128, 0:DSH],
                    start=True, stop=True,
                )

            carry_prev = carry_zero
            for k in range(NBLK):
                t0 = k * BLK
                L = min(BLK, T - t0)

                x_sb = xp.tile([128, DSH], BF16)
                nc.sync.dma_start(out=x_sb[:L, :], in_=x_d[t0 : t0 + L, :])
                tw_sb = twp.tile([128, KC], BF16)
                nc.sync.dma_start(out=tw_sb[:L, :], in_=tw_d[t0 : t0 + L, :])

                # contributions, kc-major: c[s, kc*DSH + d] = x[s,d] * tw[s,kc]
                # as ONE bf16 tensor_tensor in the DVE 2x mode. The tw operand
                # streams from a 16x-replicated tile (built by log-doubling
                # copies on the otherwise-idle GpSimd engine) through a 4-D AP
                # whose innermost dim has stride 1 -- a 0-stride dim anywhere
                # closer in would demote the op to 1x, and a per-kc
                # tensor_scalar is stuck at 1x too (its scalar operand must be
                # fp32).
                rep16 = repp.tile([128, KC * 16], BF16)
                r16v = rep16.rearrange("p (a c) -> p a c", c=16)
                nc.gpsimd.tensor_copy(r16v[:L, :, 0:1], tw_sb[:L, :, None])
                w = 1
                while w < 16:
                    nc.gpsimd.tensor_copy(r16v[:L, :, w : 2 * w], r16v[:L, :, 0:w])
                    w *= 2
                c_sb = cp.tile([128, NKC], BF16)
                c_v = c_sb[:L, :].rearrange("p (a b c) -> p a b c", b=16, c=16)
                x_v = (
                    x_sb[:L, :]
                    .rearrange("p (b c) -> p b c", c=16)
                    .unsqueeze(1)
                    .broadcast_to((L, KC, 16, 16))
                )
                rep_v = (
                    rep16[:L, :]
                    .rearrange("p (a c) -> p a c", c=16)
                    .unsqueeze(2)
                    .broadcast_to((L, KC, 16, 16))
                )
                nc.vector.tensor_mul(c_v, x_v, rep_v)
                # carry row: flattened (kc, d) sums over all previous blocks
                nc.gpsimd.dma_start(out=c_sb[L : L + 1, :], in_=carry_prev[:, :])

                # carry for the next block: += tw_k^T @ x_k
                if k + 1 < NBLK:
                    delta = pdelta.tile([KC, DSH], F32)
                    nc.tensor.matmul(
                        delta[:, :], lhsT=tw_sb[:L, :], rhs=x_sb[:L, :],
                        start=True, stop=True,
                    )
                    carry_new = carryp.tile([KC, DSH], BF16)
                    if k == 0:
                        nc.vector.tensor_copy(carry_new[:, :], delta[:, :])
                    else:
                        nc.vector.tensor_add(
                            carry_new[:, :], carry_prev[:, :], delta[:, :]
                        )
                    carry_prev = carry_new

                # all 11 eviction groups land in ONE staging tile (disjoint
                # column ranges from two engines; subtile dep tracking keeps
                # them concurrent), so the whole block ships as a single
                # contiguous ~4 MiB HWDGE store with 32 KiB descriptors

                # full 128-column stationary (cols >= L are zero-padded in
                # utri) so walrus enables FWL on the LDWEIGHTS
                lhsT = utri_sb[: L + 1, 128 * k : 128 * (k + 1)]
                og = outp.tile([128, NKC], BF16)
                for gi, (j0, gn) in enumerate(_EVICT_GROUPS):
                    pg = pmain.tile([128, 1536], F32)
                    for jj in range(gn):
                        j = j0 + jj
                        nc.tensor.matmul(
                            pg[:, jj * 512 : (jj + 1) * 512],
                            lhsT=lhsT,
                            rhs=c_sb[: L + 1, j * 512 : (j + 1) * 512],
                            start=True, stop=True,
                        )
                    col = j0 * 512
                    if gi in _DVE_GROUPS:
                        nc.vector.tensor_copy(
                            og[:L, col : col + gn * 512], pg[:L, : gn * 512]
                        )
                    else:
                        nc.scalar.copy(
                            og[:L, col : col + gn * 512], pg[:L, : gn * 512]
                        )
                # one store per block; alternate the two HW-DGE queues so
                # consecutive blocks' stores drain concurrently
                eng = nc.sync if k % 2 == 0 else nc.scalar
                eng.dma_start(out=out_d[t0 : t0 + L, :], in_=og[:L, :])
    nc.compile()
    return nc


def kernel(**inputs) -> np.ndarray:
    global LAST_RESULTS
    x = np.asarray(inputs["x"])                       # (4,1024,512) bf16
    tw = np.asarray(inputs["twiddles"])               # (1024,32,2) bf16
    pos = np.asarray(inputs["pos_norm"])              # (1024,) bf16

    tw2 = np.ascontiguousarray(tw.reshape(T, KC))
    utri = _build_utri(pos)

    in_maps = []
    for core in range(8):
        b, dh = core // 2, core % 2
        xs = np.ascontiguousarray(x[b, :, dh * DSH : (dh + 1) * DSH])
        in_maps.append({"x_shard": xs, "tw": tw2, "utri": utri})

    nc = _build_program()
    res = run_bass_kernel_spmd(nc, in_maps, core_ids=list(range(8)))
    LAST_RESULTS = res

    out = np.empty((B, T, D, KC // 2, 2), dtype=x.dtype)
    for core in range(8):
        b, dh = core // 2, core % 2
        o = np.asarray(res.results[core]["out_shard"])  # (T, NKC) kc-major
        o = o.reshape(T, KC, DSH).transpose(0, 2, 1)    # -> (T, DSH, KC)
        out[b, :, dh * DSH : (dh + 1) * DSH, :, :] = o.reshape(T, DSH, KC // 2, 2)
    return out


if __name__ == "__main__":
    rng = np.random.default_rng(0)
    demo = {
        "x": rng.standard_normal((B, T, D), np.float32).astype(ml_dtypes.bfloat16),
        "twiddles": rng.standard_normal((T, KC // 2, 2), np.float32).astype(
            ml_dtypes.bfloat16
        ),
        "pos_norm": (1.0 / np.sqrt(np.arange(1, T + 1, dtype=np.float32))).astype(
            ml_dtypes.bfloat16
        ),
    }
    print(kernel(**demo).shape)


# revision 4
# speedup vs baseline: 6.5687x; 1.2516x over previous
"""Cumulative-FFT Trainium2 kernel.

out[b,t,d,k,c] = pos_norm[t] * cumsum_t( x[b,t,d] * twiddles[t,k,c] )

Shapes (hardcoded): x (4,1024,512) bf16, twiddles (1024,32,2) bf16,
pos_norm (1024,) bf16  ->  out (4,1024,512,32,2) bf16.

Sharding: 8 cores = batch(4) x d_model-half(2). Each core computes a
(1024, 256*64) bf16 shard (32 MiB) -- data-parallel over B, tensor-parallel
over D, nothing crosses cores.

Per-core algorithm: the cumsum along t is done as a per-block triangular
matmul on the TensorEngine. t is split into 8 blocks of exactly 128 rows;
the moving operand c holds the bf16 contributions
c[s, kc*256+d] = x[s,d]*tw[s,kc] (one 2x-mode DVE tensor_tensor against a
16x-replicated tw tile). The carry (column sums of all previous blocks,
maintained by a tiny tw^T @ x matmul per block) is folded into c's row 0
by an accumulating SWDGE DMA, so the block is exactly 128 rows and the
stationary operand is a full [128,128] tile (FWL-eligible):

    utri[s, t] = pos_norm[t0+t] * (1 if s <= t else 0)

so  psum[t, n] = pos[t] * (carry[n] + sum_{s<=t} c[s, n])  comes out of the
matmul fully finished; eviction to SBUF is a pure fp32->bf16 copy split
between VectorE and ScalarE into ONE [128, 16384] staging tile per block,
then a single 4 MiB contiguous HWDGE store per block (alternating the
qSync/qScalar HW-DGE queues) writes the shard.

Hard-won trace facts this layout is built on:
 - HWDGE stripes a DMA across the 16 SDMA engines only when the partition
   count divides by 16; a 127-row store runs on ONE engine at ~27 GB/s.
   All DMAs here are 128-partition.
 - The gpsimd SWDGE queue emits ~15 eight-byte bookkeeping packets per
   data packet on table-addressed (DRAM) transfers, so all bulk traffic
   goes HWDGE; SWDGE only carries the tiny addr-immediate carry DMA.
 - DVE TENSOR_TENSOR bf16 is capped at 2x mode ((58+FD/2)/0.96GHz);
   PSUM-source evictions are capped at 1x; ScalarE (1.2 GHz) is faster
   per element for evictions, so it takes 9/11 eviction groups.
 - The PE HAM limits sustained PE utilization to ~50-55% (k=4 epochs),
   so matmul columns are the hard floor; BLK=128 minimizes column count.
"""

import sys

sys.path.insert(0, "/opt/trn_rl_repo")

import ml_dtypes
import numpy as np

import concourse.bass as bass
import concourse.mybir as mybir
import concourse.tile as tile
from concourse import bacc
import concourse.bass_utils as _bu
from concourse.bass_utils import run_bass_kernel_spmd

# note: walrus --enable-ldw-opt=true crashes codegen (visitInstLdweights),
# so the per-matmul LDWEIGHTS reload cannot be elided

B, T, D = 4, 1024, 512
KC = 64            # 32 freqs x (cos,sin), flattened innermost dims of out
DSH = D // 2       # d-slice per core
NKC = DSH * KC     # free elements per t per core (16384)
BLK = 128          # rows per t-block
NBLK = T // BLK    # 8

BF16 = mybir.dt.bfloat16
F32 = mybir.dt.float32

# groups of consecutive 512-wide matmul tiles evicted by one copy op
_EVICT_GROUPS = [(g * 3, min(3, 32 - g * 3)) for g in range((32 + 2) // 3)]
_DVE_GROUPS = (0, 5)  # eviction groups handled by VectorE (rest ScalarE)

LAST_RESULTS = None  # set by kernel(); test.py reads exec_time_ns from here


def _build_utri(pos_norm: np.ndarray) -> np.ndarray:
    """Stationary operands for all blocks, packed (128, NBLK*128) bf16."""
    pos = np.asarray(pos_norm).astype(np.float32)
    utri = np.zeros((128, NBLK * 128), np.float32)
    s = np.arange(128)[:, None]
    t = np.arange(128)[None, :]
    for k in range(NBLK):
        t0 = k * BLK
        utri[:, 128 * k : 128 * (k + 1)] = (s <= t) * pos[t0 : t0 + 128][None, :]
    return utri.astype(ml_dtypes.bfloat16)


def _build_program() -> bass.Bass:
    nc = bacc.Bacc("TRN2", target_bir_lowering=False, debug=False)
    x_d = nc.dram_tensor("x_shard", [T, DSH], BF16, kind="ExternalInput").ap()
    tw_d = nc.dram_tensor("tw", [T, KC], BF16, kind="ExternalInput").ap()
    utri_d = nc.dram_tensor("utri", [128, NBLK * 128], BF16, kind="ExternalInput").ap()
    out_d = nc.dram_tensor("out_shard", [T, NKC], BF16, kind="ExternalOutput").ap()

    with tile.TileContext(nc) as tc:
        with (
            tc.tile_pool(name="singles", bufs=1) as singles,
            tc.tile_pool(name="cp", bufs=3) as cp,
            tc.tile_pool(name="outp", bufs=2) as outp,
            tc.tile_pool(name="repp", bufs=2) as repp,
            tc.tile_pool(name="carryp", bufs=3) as carryp,
            tc.tile_pool(name="pmain", bufs=2, space="PSUM") as pmain,
            tc.tile_pool(name="pdelta", bufs=2, space="PSUM") as pdelta,
        ):
            utri_sb = singles.tile([128, NBLK * 128], BF16)
            nc.sync.dma_start(out=utri_sb[:, :], in_=utri_d[:, :])
            # whole x/tw shard loaded in one 128-partition DMA each, before
            # any store traffic enters the HWDGE queues: partition p, chunk
            # j holds row j*128+p, exactly the block layout the TT needs
            x_all = singles.tile([128, NBLK * DSH], BF16)
            nc.sync.dma_start(
                out=x_all.rearrange("p (j d) -> p j d", j=NBLK),
                in_=x_d.rearrange("(j p) d -> p j d", p=128),
            )
            tw_all = singles.tile([128, NBLK * KC], BF16)
            nc.sync.dma_start(
                out=tw_all.rearrange("p (j k) -> p j k", j=NBLK),
                in_=tw_d.rearrange("(j p) k -> p j k", p=128),
            )

            carry_prev = None
            for k in range(NBLK):
                t0 = k * BLK
                x_sb = x_all[:, k * DSH : (k + 1) * DSH]
                tw_sb = tw_all[:, k * KC : (k + 1) * KC]

                # contributions, kc-major: c[s, kc*DSH + d] = x[s,d] * tw[s,kc]
                # as ONE bf16 tensor_tensor in the DVE 2x mode. The tw operand
                # streams from a 16x-replicated tile (built by log-doubling
                # copies on the otherwise-idle GpSimd engine) through a 4-D AP
                # whose innermost dim has stride 1 -- a 0-stride dim anywhere
                # closer in would demote the op to 1x, and a per-kc
                # tensor_scalar is stuck at 1x too (its scalar operand must be
                # fp32).
                rep16 = repp.tile([128, KC * 16], BF16)
                r16v = rep16.rearrange("p (a c) -> p a c", c=16)
                nc.gpsimd.tensor_copy(r16v[:, :, 0:1], tw_sb[:, :, None])
                w = 1
                while w < 16:
                    nc.gpsimd.tensor_copy(r16v[:, :, w : 2 * w], r16v[:, :, 0:w])
                    w *= 2
                c_sb = cp.tile([128, NKC], BF16)
                c_v = c_sb.rearrange("p (a b c) -> p a b c", b=16, c=16)
                x_v = (
                    x_sb
                    .rearrange("p (b c) -> p b c", c=16)
                    .unsqueeze(1)
                    .broadcast_to((128, KC, 16, 16))
                )
                rep_v = (
                    rep16[:, :]
                    .rearrange("p (a c) -> p a c", c=16)
                    .unsqueeze(2)
                    .broadcast_to((128, KC, 16, 16))
                )
                nc.vector.tensor_mul(c_v, x_v, rep_v)
                # fold the carry (column sums of all previous blocks) into
                # c's first row: SWDGE DMA with inline CCE add. Both sides
                # are addr-immediate SBUF, which keeps the SWDGE descriptor
                # path clean (no 8-byte table-read packet storm).
                if carry_prev is not None:
                    nc.gpsimd.dma_start(
                        out=c_sb[0:1, :], in_=carry_prev[:, :],
                        accum_op=mybir.AluOpType.add,
                    )

                # carry for the next block: += tw_k^T @ x_k
                if k + 1 < NBLK:
                    delta = pdelta.tile([KC, DSH], F32)
                    nc.tensor.matmul(
                        delta[:, :], lhsT=tw_sb, rhs=x_sb,
                        start=True, stop=True,
                    )
                    carry_new = carryp.tile([KC, DSH], BF16)
                    if k == 0:
                        nc.vector.tensor_copy(carry_new[:, :], delta[:, :])
                    else:
                        nc.vector.tensor_add(
                            carry_new[:, :], carry_prev[:, :], delta[:, :]
                        )
                    carry_prev = carry_new

                # all 11 eviction groups land in ONE staging tile (disjoint
                # column ranges from two engines; subtile dep tracking keeps
                # them concurrent), so the whole block ships as a single
                # contiguous 4 MiB HWDGE store with 32 KiB descriptors
                lhsT = utri_sb[:, 128 * k : 128 * (k + 1)]
                og = outp.tile([128, NKC], BF16)
                for gi, (j0, gn) in enumerate(_EVICT_GROUPS):
                    pg = pmain.tile([128, 1536], F32)
                    for jj in range(gn):
                        j = j0 + jj
                        nc.tensor.matmul(
                            pg[:, jj * 512 : (jj + 1) * 512],
                            lhsT=lhsT,
                            rhs=c_sb[:, j * 512 : (j + 1) * 512],
                            start=True, stop=True,
                        )
                    col = j0 * 512
                    if gi in _DVE_GROUPS:
                        nc.vector.tensor_copy(
                            og[:, col : col + gn * 512], pg[:, : gn * 512]
                        )
                    else:
                        nc.scalar.copy(
                            og[:, col : col + gn * 512], pg[:, : gn * 512]
                        )
                # one store per block; alternate the two HW-DGE queues so
                # consecutive blocks' stores drain concurrently
                eng = nc.sync if k % 2 == 0 else nc.scalar
                eng.dma_start(out=out_d[t0 : t0 + BLK, :], in_=og[:, :])
    nc.compile()
    return nc


def kernel(**inputs) -> np.ndarray:
    global LAST_RESULTS
    x = np.asarray(inputs["x"])                       # (4,1024,512) bf16
    tw = np.asarray(inputs["twiddles"])               # (1024,32,2) bf16
    pos = np.asarray(inputs["pos_norm"])              # (1024,) bf16

    tw2 = np.ascontiguousarray(tw.reshape(T, KC))
    utri = _build_utri(pos)

    in_maps = []
    for core in range(8):
        b, dh = core // 2, core % 2
        xs = np.ascontiguousarray(x[b, :, dh * DSH : (dh + 1) * DSH])
        in_maps.append({"x_shard": xs, "tw": tw2, "utri": utri})

    nc = _build_program()
    res = run_bass_kernel_spmd(nc, in_maps, core_ids=list(range(8)))
    LAST_RESULTS = res

    out = np.empty((B, T, D, KC // 2, 2), dtype=x.dtype)
    for core in range(8):
        b, dh = core // 2, core % 2
        o = np.asarray(res.results[core]["out_shard"])  # (T, NKC) kc-major
        o = o.reshape(T, KC, DSH).transpose(0, 2, 1)    # -> (T, DSH, KC)
        out[b, :, dh * DSH : (dh + 1) * DSH, :, :] = o.reshape(T, DSH, KC // 2, 2)
    return out


if __name__ == "__main__":
    rng = np.random.default_rng(0)
    demo = {
        "x": rng.standard_normal((B, T, D), np.float32).astype(ml_dtypes.bfloat16),
        "twiddles": rng.standard_normal((T, KC // 2, 2), np.float32).astype(
            ml_dtypes.bfloat16
        ),
        "pos_norm": (1.0 / np.sqrt(np.arange(1, T + 1, dtype=np.float32))).astype(
            ml_dtypes.bfloat16
        ),
    }
    print(kernel(**demo).shape)
